# revision 6
# baseline (speedup 1.0000x reference)
"""MoE expert-parallel kernel v5 for Trainium2 (Bass/Tile).

8 experts, 8 NeuronCores, one expert per core (SPMD, no collectives).
Per core: out = gelu(x @ w1) @ w2 with
  x [2048, 1024] f32, w1 [1024, 4096] f32, w2 [4096, 1024] f32.

Design (v3 baseline 546.4 us -> ~542 us; the kernel is at the HW roofline):
  - HW ablations showed the stream is purely PE-rate bound. With all 8 cores
    active the PE sustains only ~1.95 GHz (P0 power downclock; the same
    program runs 216 ns/MM = 2.4 GHz on one active core, 265 ns/MM on 8).
    2048 N=512 matmuls x ~265 ns ~= 542 us is the power-limited floor;
    PE busy is ~100% (1-core run: 443 us vs 437 us theoretical peak).
  - All inputs host-prepped into exact SBUF tile layouts: w1/w2 bf16, x
    fp8e3 (e3m4; one-sided quantization of GEMM1's moving operand costs
    ~1.4% rel err vs the 2e-2 gate and trims stream bytes/power). Every
    device DMA is a plain contiguous HWDGE copy; no on-device casts or
    transposes. Output leaves as bf16 outT [1024, 2048]; host finishes.
  - Both GEMMs weight-stationary with the full 2048-token dim moving:
    each stationary tile feeds 4 matmuls (4 token spans of 512 -> 4 PSUM
    banks). A BIR post-pass deletes the redundant Ldweights (walrus emits
    LDW+MM 1:1 with ldweights:false on the MM), so 2048 MMs carry only 512
    weight loads, the structural minimum (contraction_tiles x out_tiles).
  - GEMM1: for j in 32: chain k=0..7 on stationary w1[k,j], moving
    xT[k, tok]; GELU evicts 4 banks -> resident ht[j] (bf16, n-major,
    128 KB/partition). GEMM2: for d in 8: chain j=0..31 on stationary
    w2[j,d], moving ht[j] -> outT staged bf16 + SWDGE store (keeps the
    sync HWDGE ring weights-only so the next rep's w1 is never queued
    behind an out-store).
  - Streams: sync ring w1 (8 chunks) + w2 (8 chunks); scalar ring xt;
    gpsimd out. DMA is fully hidden (noreload ablation == full time).
"""

import os
import sys

import numpy as np

if os.path.isdir("/opt/trn_rl_repo") and "/opt/trn_rl_repo" not in sys.path:
    sys.path.insert(0, "/opt/trn_rl_repo")

NUM_EXPERTS = 8
TOK = 2048
HID = 1024
INT = 4096
OUT = 1024
P = 128

KT = HID // P        # 8 k-tiles (GEMM1 contraction)
JT = INT // P        # 32 j-tiles (intermediate)
DT = OUT // P        # 8 d-tiles (output)
NS = TOK // 512      # 4 moving token spans of 512

W1CJ = 4             # j-tiles per streamed w1 chunk
W1C = JT // W1CJ     # 8 w1 chunks per rep

# GEMM1's moving operand (xT) is fp8e3 (e3m4): one-sided quantization costs
# ~1.4% rel err (vs the 2e-2 gate) and halves the moving-stream bytes /
# reduces PE switching power, which matters because the 8-core kernel is
# P0-power-downclock-bound (~2.0 GHz vs 2.4 single-core).
X_FP8 = True

_PROGRAM_CACHE = {}

# ---------------------------------------------------------------------------
# BIR post-passes, applied via a compile hook:
#  1) dedup_ldweights: walrus emits one Ldweights per Matmult (the Matmult
#     itself has ldweights:false). Consecutive Ldweights with identical
#     weight APs reload the same stationary operand; drop the repeats
#     (convert to NoOp when they carry sync_info).
#  2) split_excess_waits: this compiler build encodes at most 1 sem-wait per
#     instruction; move extras onto NoOps placed before the offender.

LDW_DEDUP = True


def _dedup_ldweights(d) -> bool:
    import orjson

    changed = False
    for fn in d.get("functions", []):
        for blk in fn.get("blocks", []):
            last_key = None
            out = []
            for ins in blk.get("instructions", []):
                if ins.get("engine") != "PE":
                    out.append(ins)
                    continue
                op = ins.get("opcode")
                if op == "Ldweights":
                    key = orjson.dumps(
                        (
                            ins.get("ins"),
                            ins.get("tile_position"),
                            ins.get("tile_size"),
                            ins.get("perf_mode"),
                            ins.get("is_transpose"),
                        )
                    )
                    if key == last_key:
                        changed = True
                        sync = ins.get("sync_info") or {}
                        if sync.get("on_wait") or sync.get("on_update"):
                            ins["opcode"] = "NoOp"
                            ins["ins"] = []
                            ins["outs"] = []
                            out.append(ins)
                        # else: drop entirely
                    else:
                        last_key = key
                        out.append(ins)
                elif op in ("Matmult", "NoOp"):
                    out.append(ins)
                else:
                    last_key = None
                    out.append(ins)
            blk["instructions"] = out
    return changed


def _split_excess_waits(d) -> bool:
    changed = False
    for fn in d.get("functions", []):
        for blk in fn.get("blocks", []):
            out = []
            for ins in blk.get("instructions", []):
                sync = ins.get("sync_info")
                waits = (sync or {}).get("on_wait") or []
                cap = 1
                if len(waits) > cap:
                    changed = True
                    extra, keep = waits[:-cap], waits[-cap:]
                    for i in range(len(extra)):
                        out.append({
                            "name": f"{ins['name']}-wsplit{i}",
                            "opcode": "NoOp",
                            "engine": ins["engine"],
                            "ins": [],
                            "outs": [],
                            "debug": ins.get("debug", 0),
                            "sync_info": {"on_update": [], "on_wait": [extra[i]]},
                        })
                    sync["on_wait"] = keep
                out.append(ins)
            blk["instructions"] = out
    return changed


def _bir_postprocess(bir_json: bytes) -> bytes:
    import orjson

    d = orjson.loads(bir_json)
    changed = False
    if LDW_DEDUP:
        changed |= _dedup_ldweights(d)
    changed |= _split_excess_waits(d)
    return orjson.dumps(d) if changed else bir_json


_hook_installed = False


def _install_wait_split_hook():
    global _hook_installed
    if _hook_installed:
        return
    import concourse.bass2jax as bass2jax
    import concourse.bass_utils as bass_utils

    orig = bass_utils.compile_bir_kernel

    def patched(bir_json, tmpdir, neff_name="file.neff"):
        return orig(_bir_postprocess(bir_json), tmpdir, neff_name)

    bass2jax.compile_bir_kernel = patched
    bass_utils.compile_bir_kernel = patched
    _hook_installed = True


# ---------------------------------------------------------------------------
# Host-side input/output prep


def prep_expert(x, w1, w2):
    """Cast one expert's tensors to bf16 in device tile layout.

    xt  [128, KT*2048]: xt[p, kt, t] = x[t, kt*128 + p]
    w1h [128, JT*KT*128]: w1h[p, jt, kt, c] = w1[kt*128 + p, jt*128 + c]
    w2h [128, DT*JT*128]: w2h[p, dt, jt, c] = w2[jt*128 + p, dt*128 + c]
    """
    import ml_dtypes

    bf = ml_dtypes.bfloat16
    if X_FP8:
        xb = np.clip(x, -15.0, 15.0).astype(ml_dtypes.float8_e3m4)
    else:
        xb = x.astype(bf)
    w1b = w1.astype(bf)
    w2b = w2.astype(bf)
    xt = np.ascontiguousarray(
        xb.T.reshape(KT, P, TOK).transpose(1, 0, 2)
    ).reshape(P, KT * TOK)
    w1h = np.ascontiguousarray(
        w1b.reshape(KT, P, JT, P).transpose(1, 2, 0, 3)
    ).reshape(P, JT * KT * P)
    w2h = np.ascontiguousarray(
        w2b.reshape(JT, P, DT, P).transpose(1, 2, 0, 3)
    ).reshape(P, DT * JT * P)
    return {"xt": xt, "w1": w1h, "w2": w2h}


def make_in_maps(x, w1, w2):
    x = np.ascontiguousarray(x, dtype=np.float32)
    w1 = np.ascontiguousarray(w1, dtype=np.float32)
    w2 = np.ascontiguousarray(w2, dtype=np.float32)
    assert x.shape == (NUM_EXPERTS, TOK, HID)
    assert w1.shape == (NUM_EXPERTS, HID, INT)
    assert w2.shape == (NUM_EXPERTS, INT, OUT)
    return [prep_expert(x[e], w1[e], w2[e]) for e in range(NUM_EXPERTS)]


def finish_output(raw):
    """raw [128, DT*2048] bf16 with raw[p, dt, t] = out[t, dt*128 + p]."""
    arr = np.asarray(raw).reshape(P, DT, TOK).transpose(2, 1, 0)
    return np.ascontiguousarray(arr.reshape(TOK, OUT), dtype=np.float32)


# ---------------------------------------------------------------------------
# Program


def build_program(key=None, repeats=1, ablate=None):
    import concourse.bass as bass
    import concourse.tile as tile
    from concourse import mybir

    if ablate is None:
        ablate = ""
    ablate = set(a for a in ablate.split(",") if a)

    f32 = mybir.dt.float32
    bf16 = mybir.dt.bfloat16
    xdt = mybir.dt.float8e3 if X_FP8 else bf16

    nc = bass.Bass()
    xt_h = nc.declare_dram_parameter("xt", [P, KT * TOK], xdt, isOutput=False)
    w1_h = nc.declare_dram_parameter("w1", [P, JT * KT * P], bf16, isOutput=False)
    w2_h = nc.declare_dram_parameter("w2", [P, DT * JT * P], bf16, isOutput=False)
    out_h = nc.declare_dram_parameter("out", [P, DT * TOK], bf16, isOutput=True)

    xt_r = xt_h[:, :].rearrange("p (kt t) -> p kt t", t=TOK)
    w1_r = w1_h[:, :].rearrange("p (jt kt c) -> p jt kt c", kt=KT, c=P)
    w2_r = w2_h[:, :].rearrange("p (dt jt c) -> p dt jt c", jt=JT, c=P)
    out_r = out_h[:, :].rearrange("p (dt t) -> p dt t", t=TOK)

    gelu = mybir.ActivationFunctionType.Gelu

    with tile.TileContext(nc) as tc:
        with (
            tc.tile_pool(name="xtp", bufs=1) as xt_pool,
            tc.tile_pool(name="htp", bufs=1) as ht_pool,
            tc.tile_pool(name="w1p", bufs=2) as w1_pool,
            tc.tile_pool(name="w2p", bufs=2) as w2_pool,
            tc.tile_pool(name="ostg", bufs=2) as ost_pool,
            tc.tile_pool(name="ps", bufs=2, space="PSUM") as ps_pool,
        ):
            pools = (xt_pool, ht_pool, w1_pool, w2_pool, ost_pool, ps_pool)
            pre = {}
            if "nog1" in ablate:
                ht = ht_pool.tile([P, JT, TOK], bf16, name="ht")
                nc.gpsimd.memset(ht[:], 0.25)
                pre["ht"] = ht
            for _rep in range(repeats):
                _emit_rep(nc, f32, bf16, gelu, xt_r, w1_r, w2_r, out_r,
                          pools, ablate, pre)
    return nc


def _emit_rep(nc, f32, bf16, gelu, xt_r, w1_r, w2_r, out_r, pools,
              ablate=frozenset(), pre=None):
    (xt_pool, ht_pool, w1_pool, w2_pool, ost_pool, ps_pool) = pools
    pre = pre or {}
    skip_g1 = "nog1" in ablate
    skip_g2 = "nog2" in ablate

    # ---- GEMM1: ht[j, tok] = gelu(w1.T @ xT) --------------------------
    if skip_g1:
        ht = pre["ht"]
    else:
        ht = ht_pool.tile([P, JT, TOK], bf16, name="ht")
        xdt = xt_r.dtype
        xt = xt_pool.tile([P, KT, TOK], xdt, name="xt")
        # xt load on the scalar HWDGE ring (weights go on sync)
        nc.scalar.dma_start(out=xt[:, :KT // 2, :], in_=xt_r[:, :KT // 2, :])
        nc.scalar.dma_start(out=xt[:, KT // 2:, :], in_=xt_r[:, KT // 2:, :])

        w1c = None
        for j in range(JT):
            if j % W1CJ == 0:
                w1c = w1_pool.tile([P, W1CJ, KT, P], bf16, name="w1c")
                nc.sync.dma_start(
                    out=w1c[:],
                    in_=w1_r[:, j:j + W1CJ, :, :],
                )
            jj = j % W1CJ
            hp = ps_pool.tile([P, NS, 512], f32, name="acc")
            for k in range(KT):
                lhs = w1c[:, jj, k, :]
                for s in range(NS):
                    nc.tensor.matmul(
                        hp[:, s, :],
                        lhs,
                        xt[:, k, s * 512:(s + 1) * 512],
                        start=(k == 0),
                        stop=(k == KT - 1),
                    )
            nc.scalar.activation(ht[:, j, :], hp[:, :, :], gelu)

    # ---- GEMM2: outT[d, tok] = ht.T-contract @ w2 ---------------------
    if skip_g2:
        # still evict something so the rep has an output dependency
        ot = ost_pool.tile([P, TOK], bf16, name="ot")
        nc.scalar.copy(ot[:], ht[:, 0, :])
        nc.sync.dma_start(out=out_r[:, 0, :], in_=ot[:])
        return

    for d in range(DT):
        w2c = w2_pool.tile([P, 1, JT, P], bf16, name="w2c")
        nc.sync.dma_start(out=w2c[:], in_=w2_r[:, d:d + 1, :, :])
        op = ps_pool.tile([P, NS, 512], f32, name="acc")
        for j in range(JT):
            lhs = w2c[:, 0, j, :]
            for s in range(NS):
                nc.tensor.matmul(
                    op[:, s, :],
                    lhs,
                    ht[:, j, s * 512:(s + 1) * 512],
                    start=(j == 0),
                    stop=(j == JT - 1),
                )
        if "noout" in ablate and d != DT - 1:
            continue
        ot = ost_pool.tile([P, NS, 512], bf16, name="ot")
        nc.scalar.copy(ot[:], op[:])
        # out-store on the (otherwise idle) SWDGE path: keeps the sync HWDGE
        # ring weights-only, so the next rep's first w1 chunk is not queued
        # behind this rep's last out-store (FIFO per ring).
        nc.gpsimd.dma_start(
            out=out_r[:, d, :],
            in_=ot[:].rearrange("p a b -> p (a b)"),
        )


def _get_program():
    if "v4" not in _PROGRAM_CACHE:
        _PROGRAM_CACHE["v4"] = build_program()
    return _PROGRAM_CACHE["v4"]


def kernel(x, w1, w2, _trace=False, _trace_kwargs=None):
    """Full-tensor entry point: shards experts across 8 cores, returns full out."""
    from concourse.bass_utils import run_bass_kernel_spmd

    _install_wait_split_hook()
    in_maps = make_in_maps(x, w1, w2)
    nc = _get_program()
    core_ids = list(range(NUM_EXPERTS))
    kw = {}
    if _trace:
        kw["trace"] = True
        kw["trace_kwargs"] = _trace_kwargs or {}
    res = run_bass_kernel_spmd(nc, in_maps, core_ids, **kw)
    out = np.stack(
        [finish_output(res.results[e]["out"]) for e in range(NUM_EXPERTS)], axis=0
    )
    if _trace:
        return out, res
    return out


if __name__ == "__main__":
    rng = np.random.default_rng(0)
    x = rng.standard_normal((NUM_EXPERTS, TOK, HID), dtype=np.float32)
    w1 = rng.standard_normal((NUM_EXPERTS, HID, INT), dtype=np.float32) * 0.03
    w2 = rng.standard_normal((NUM_EXPERTS, INT, OUT), dtype=np.float32) * 0.015
    out = kernel(x, w1, w2)
    print("out", out.shape, out.dtype, float(np.abs(out).mean()))


# revision 7
# speedup vs baseline: 1.0024x; 1.0024x over previous
"""MoE expert-parallel kernel v5 for Trainium2 (Bass/Tile).

8 experts, 8 NeuronCores, one expert per core (SPMD, no collectives).
Per core: out = gelu(x @ w1) @ w2 with
  x [2048, 1024] f32, w1 [1024, 4096] f32, w2 [4096, 1024] f32.

Design (v3 baseline 546.4 us -> ~542 us; the kernel is at the HW roofline):
  - HW ablations showed the stream is purely PE-rate bound. With all 8 cores
    active the PE sustains only ~1.95 GHz (P0 power downclock; the same
    program runs 216 ns/MM = 2.4 GHz on one active core, 265 ns/MM on 8).
    2048 N=512 matmuls x ~265 ns ~= 542 us is the power-limited floor;
    PE busy is ~100% (1-core run: 443 us vs 437 us theoretical peak).
  - All inputs host-prepped into exact SBUF tile layouts: w1/w2 bf16, x
    fp8e3 (e3m4; one-sided quantization of GEMM1's moving operand costs
    ~1.4% rel err vs the 2e-2 gate and trims stream bytes/power). Every
    device DMA is a plain contiguous HWDGE copy; no on-device casts or
    transposes. Output leaves as bf16 outT [1024, 2048]; host finishes.
  - Both GEMMs weight-stationary with the full 2048-token dim moving:
    each stationary tile feeds 4 matmuls (4 token spans of 512 -> 4 PSUM
    banks). A BIR post-pass deletes the redundant Ldweights (walrus emits
    LDW+MM 1:1 with ldweights:false on the MM), so 2048 MMs carry only 512
    weight loads, the structural minimum (contraction_tiles x out_tiles).
  - GEMM1: for j in 32: chain k=0..7 on stationary w1[k,j], moving
    xT[k, tok]; GELU evicts 4 banks -> resident ht[j] (bf16, n-major,
    128 KB/partition). GEMM2: for d in 8: chain j=0..31 on stationary
    w2[j,d], moving ht[j] -> outT staged bf16 + SWDGE store (keeps the
    sync HWDGE ring weights-only so the next rep's w1 is never queued
    behind an out-store).
  - Streams: sync ring w1 (8 chunks) + w2 (8 chunks); scalar ring xt;
    gpsimd out. DMA is fully hidden (noreload ablation == full time).
"""

import os
import sys

import numpy as np

if os.path.isdir("/opt/trn_rl_repo") and "/opt/trn_rl_repo" not in sys.path:
    sys.path.insert(0, "/opt/trn_rl_repo")

NUM_EXPERTS = 8
TOK = 2048
HID = 1024
INT = 4096
OUT = 1024
P = 128

KT = HID // P        # 8 k-tiles (GEMM1 contraction)
JT = INT // P        # 32 j-tiles (intermediate)
DT = OUT // P        # 8 d-tiles (output)
NS = TOK // 512      # 4 moving token spans of 512

W1CJ = 4             # j-tiles per streamed w1 chunk
W1C = JT // W1CJ     # 8 w1 chunks per rep

# GEMM1's moving operand (xT) is fp8e3 (e3m4): one-sided quantization costs
# ~1.4% rel err (vs the 2e-2 gate) and halves the moving-stream bytes /
# reduces PE switching power, which matters because the 8-core kernel is
# P0-power-downclock-bound (~2.0 GHz vs 2.4 single-core).
X_FP8 = True

_PROGRAM_CACHE = {}

# ---------------------------------------------------------------------------
# BIR post-passes, applied via a compile hook:
#  1) dedup_ldweights: walrus emits one Ldweights per Matmult (the Matmult
#     itself has ldweights:false). Consecutive Ldweights with identical
#     weight APs reload the same stationary operand; drop the repeats
#     (convert to NoOp when they carry sync_info).
#  2) split_excess_waits: this compiler build encodes at most 1 sem-wait per
#     instruction; move extras onto NoOps placed before the offender.

LDW_DEDUP = True


def _dedup_ldweights(d) -> bool:
    import orjson

    changed = False
    for fn in d.get("functions", []):
        for blk in fn.get("blocks", []):
            last_key = None
            out = []
            for ins in blk.get("instructions", []):
                if ins.get("engine") != "PE":
                    out.append(ins)
                    continue
                op = ins.get("opcode")
                if op == "Ldweights":
                    key = orjson.dumps(
                        (
                            ins.get("ins"),
                            ins.get("tile_position"),
                            ins.get("tile_size"),
                            ins.get("perf_mode"),
                            ins.get("is_transpose"),
                        )
                    )
                    if key == last_key:
                        changed = True
                        sync = ins.get("sync_info") or {}
                        if sync.get("on_wait") or sync.get("on_update"):
                            ins["opcode"] = "NoOp"
                            ins["ins"] = []
                            ins["outs"] = []
                            out.append(ins)
                        # else: drop entirely
                    else:
                        last_key = key
                        out.append(ins)
                elif op in ("Matmult", "NoOp"):
                    out.append(ins)
                else:
                    last_key = None
                    out.append(ins)
            blk["instructions"] = out
    return changed


def _split_excess_waits(d) -> bool:
    changed = False
    for fn in d.get("functions", []):
        for blk in fn.get("blocks", []):
            out = []
            for ins in blk.get("instructions", []):
                sync = ins.get("sync_info")
                waits = (sync or {}).get("on_wait") or []
                cap = 1
                if len(waits) > cap:
                    changed = True
                    extra, keep = waits[:-cap], waits[-cap:]
                    for i in range(len(extra)):
                        out.append({
                            "name": f"{ins['name']}-wsplit{i}",
                            "opcode": "NoOp",
                            "engine": ins["engine"],
                            "ins": [],
                            "outs": [],
                            "debug": ins.get("debug", 0),
                            "sync_info": {"on_update": [], "on_wait": [extra[i]]},
                        })
                    sync["on_wait"] = keep
                out.append(ins)
            blk["instructions"] = out
    return changed


def _bir_postprocess(bir_json: bytes) -> bytes:
    import orjson

    d = orjson.loads(bir_json)
    changed = False
    if LDW_DEDUP:
        changed |= _dedup_ldweights(d)
    changed |= _split_excess_waits(d)
    return orjson.dumps(d) if changed else bir_json


_hook_installed = False


def _install_wait_split_hook():
    global _hook_installed
    if _hook_installed:
        return
    import concourse.bass2jax as bass2jax
    import concourse.bass_utils as bass_utils

    orig = bass_utils.compile_bir_kernel

    def patched(bir_json, tmpdir, neff_name="file.neff"):
        return orig(_bir_postprocess(bir_json), tmpdir, neff_name)

    bass2jax.compile_bir_kernel = patched
    bass_utils.compile_bir_kernel = patched
    _hook_installed = True


# ---------------------------------------------------------------------------
# Host-side input/output prep


def prep_expert(x, w1, w2):
    """Cast one expert's tensors to bf16 in device tile layout.

    xt  [128, KT*2048]: xt[p, kt, t] = x[t, kt*128 + p]
    w1h [128, JT*KT*128]: w1h[p, jt, kt, c] = w1[kt*128 + p, jt*128 + c]
    w2h [128, DT*JT*128]: w2h[p, dt, jt, c] = w2[jt*128 + p, dt*128 + c]
    """
    import ml_dtypes

    bf = ml_dtypes.bfloat16
    if X_FP8:
        xb = np.clip(x, -15.0, 15.0).astype(ml_dtypes.float8_e3m4)
    else:
        xb = x.astype(bf)
    w1b = w1.astype(bf)
    w2b = w2.astype(bf)
    xt = np.ascontiguousarray(
        xb.T.reshape(KT, P, TOK).transpose(1, 0, 2)
    ).reshape(P, KT * TOK)
    w1h = np.ascontiguousarray(
        w1b.reshape(KT, P, JT, P).transpose(1, 2, 0, 3)
    ).reshape(P, JT * KT * P)
    w2h = np.ascontiguousarray(
        w2b.reshape(JT, P, DT, P).transpose(1, 2, 0, 3)
    ).reshape(P, DT * JT * P)
    return {"xt": xt, "w1": w1h, "w2": w2h}


def make_in_maps(x, w1, w2):
    x = np.ascontiguousarray(x, dtype=np.float32)
    w1 = np.ascontiguousarray(w1, dtype=np.float32)
    w2 = np.ascontiguousarray(w2, dtype=np.float32)
    assert x.shape == (NUM_EXPERTS, TOK, HID)
    assert w1.shape == (NUM_EXPERTS, HID, INT)
    assert w2.shape == (NUM_EXPERTS, INT, OUT)
    return [prep_expert(x[e], w1[e], w2[e]) for e in range(NUM_EXPERTS)]


def finish_output(raw):
    """raw [128, DT*2048] bf16 with raw[p, dt, t] = out[t, dt*128 + p]."""
    arr = np.asarray(raw).reshape(P, DT, TOK).transpose(2, 1, 0)
    return np.ascontiguousarray(arr.reshape(TOK, OUT), dtype=np.float32)


# ---------------------------------------------------------------------------
# Program


def build_program(key=None, repeats=1, ablate=None):
    import concourse.bass as bass
    import concourse.tile as tile
    from concourse import mybir

    if ablate is None:
        ablate = ""
    ablate = set(a for a in ablate.split(",") if a)

    f32 = mybir.dt.float32
    bf16 = mybir.dt.bfloat16
    xdt = mybir.dt.float8e3 if X_FP8 else bf16

    nc = bass.Bass()
    xt_h = nc.declare_dram_parameter("xt", [P, KT * TOK], xdt, isOutput=False)
    w1_h = nc.declare_dram_parameter("w1", [P, JT * KT * P], bf16, isOutput=False)
    w2_h = nc.declare_dram_parameter("w2", [P, DT * JT * P], bf16, isOutput=False)
    out_h = nc.declare_dram_parameter("out", [P, DT * TOK], bf16, isOutput=True)

    xt_r = xt_h[:, :].rearrange("p (kt t) -> p kt t", t=TOK)
    w1_r = w1_h[:, :].rearrange("p (jt kt c) -> p jt kt c", kt=KT, c=P)
    w2_r = w2_h[:, :].rearrange("p (dt jt c) -> p dt jt c", jt=JT, c=P)
    out_r = out_h[:, :].rearrange("p (dt t) -> p dt t", t=TOK)

    gelu = mybir.ActivationFunctionType.Gelu

    with tile.TileContext(nc) as tc:
        with (
            tc.tile_pool(name="xtp", bufs=1) as xt_pool,
            tc.tile_pool(name="htp", bufs=1) as ht_pool,
            tc.tile_pool(name="w1p", bufs=2) as w1_pool,
            tc.tile_pool(name="w2p", bufs=2) as w2_pool,
            tc.tile_pool(name="ostg", bufs=2) as ost_pool,
            tc.tile_pool(name="ps", bufs=2, space="PSUM") as ps_pool,
        ):
            pools = (xt_pool, ht_pool, w1_pool, w2_pool, ost_pool, ps_pool)
            pre = {}
            if "nog1" in ablate:
                ht = ht_pool.tile([P, JT, TOK], bf16, name="ht")
                nc.gpsimd.memset(ht[:], 0.25)
                pre["ht"] = ht
            for _rep in range(repeats):
                _emit_rep(nc, f32, bf16, gelu, xt_r, w1_r, w2_r, out_r,
                          pools, ablate, pre)
    return nc


def _emit_rep(nc, f32, bf16, gelu, xt_r, w1_r, w2_r, out_r, pools,
              ablate=frozenset(), pre=None):
    (xt_pool, ht_pool, w1_pool, w2_pool, ost_pool, ps_pool) = pools
    pre = pre or {}
    skip_g1 = "nog1" in ablate
    skip_g2 = "nog2" in ablate

    # ---- GEMM1: ht[j, tok] = gelu(w1.T @ xT) --------------------------
    if skip_g1:
        ht = pre["ht"]
    else:
        ht = ht_pool.tile([P, JT, TOK], bf16, name="ht")
        xdt = xt_r.dtype
        xt = xt_pool.tile([P, KT, TOK], xdt, name="xt")
        # xt load on the sync HWDGE ring, ahead of the w1/w2 chunks: the
        # next rep's xt then prefetches mid-GEMM2. On the scalar ring it sat
        # behind this rep's last out-copy (which retires only at rep end),
        # serializing the 2MB load into the rep boundary.
        nc.sync.dma_start(out=xt[:, :KT // 2, :], in_=xt_r[:, :KT // 2, :])
        nc.sync.dma_start(out=xt[:, KT // 2:, :], in_=xt_r[:, KT // 2:, :])

        w1c = None
        for j in range(JT):
            if j % W1CJ == 0:
                w1c = w1_pool.tile([P, W1CJ, KT, P], bf16, name="w1c")
                nc.sync.dma_start(
                    out=w1c[:],
                    in_=w1_r[:, j:j + W1CJ, :, :],
                )
            jj = j % W1CJ
            hp = ps_pool.tile([P, NS, 512], f32, name="acc")
            for k in range(KT):
                lhs = w1c[:, jj, k, :]
                for s in range(NS):
                    nc.tensor.matmul(
                        hp[:, s, :],
                        lhs,
                        xt[:, k, s * 512:(s + 1) * 512],
                        start=(k == 0),
                        stop=(k == KT - 1),
                    )
            nc.scalar.activation(ht[:, j, :], hp[:, :, :], gelu)

    # ---- GEMM2: outT[d, tok] = ht.T-contract @ w2 ---------------------
    if skip_g2:
        # still evict something so the rep has an output dependency
        ot = ost_pool.tile([P, TOK], bf16, name="ot")
        nc.scalar.copy(ot[:], ht[:, 0, :])
        nc.sync.dma_start(out=out_r[:, 0, :], in_=ot[:])
        return

    for d in range(DT):
        w2c = w2_pool.tile([P, 1, JT, P], bf16, name="w2c")
        nc.sync.dma_start(out=w2c[:], in_=w2_r[:, d:d + 1, :, :])
        op = ps_pool.tile([P, NS, 512], f32, name="acc")
        for j in range(JT):
            lhs = w2c[:, 0, j, :]
            for s in range(NS):
                nc.tensor.matmul(
                    op[:, s, :],
                    lhs,
                    ht[:, j, s * 512:(s + 1) * 512],
                    start=(j == 0),
                    stop=(j == JT - 1),
                )
        if "noout" in ablate and d != DT - 1:
            continue
        ot = ost_pool.tile([P, NS, 512], bf16, name="ot")
        nc.scalar.copy(ot[:], op[:])
        # out-store on the (otherwise idle) SWDGE path: keeps the sync HWDGE
        # ring weights-only, so the next rep's first w1 chunk is not queued
        # behind this rep's last out-store (FIFO per ring).
        nc.gpsimd.dma_start(
            out=out_r[:, d, :],
            in_=ot[:].rearrange("p a b -> p (a b)"),
        )


def _get_program():
    if "v4" not in _PROGRAM_CACHE:
        _PROGRAM_CACHE["v4"] = build_program()
    return _PROGRAM_CACHE["v4"]


def kernel(x, w1, w2, _trace=False, _trace_kwargs=None):
    """Full-tensor entry point: shards experts across 8 cores, returns full out."""
    from concourse.bass_utils import run_bass_kernel_spmd

    _install_wait_split_hook()
    in_maps = make_in_maps(x, w1, w2)
    nc = _get_program()
    core_ids = list(range(NUM_EXPERTS))
    kw = {}
    if _trace:
        kw["trace"] = True
        kw["trace_kwargs"] = _trace_kwargs or {}
    res = run_bass_kernel_spmd(nc, in_maps, core_ids, **kw)
    out = np.stack(
        [finish_output(res.results[e]["out"]) for e in range(NUM_EXPERTS)], axis=0
    )
    if _trace:
        return out, res
    return out


if __name__ == "__main__":
    rng = np.random.default_rng(0)
    x = rng.standard_normal((NUM_EXPERTS, TOK, HID), dtype=np.float32)
    w1 = rng.standard_normal((NUM_EXPERTS, HID, INT), dtype=np.float32) * 0.03
    w2 = rng.standard_normal((NUM_EXPERTS, INT, OUT), dtype=np.float32) * 0.015
    out = kernel(x, w1, w2)
    print("out", out.shape, out.dtype, float(np.abs(out).mean()))


# revision 8
# speedup vs baseline: 1.0073x; 1.0049x over previous
"""MoE expert-parallel kernel v5 for Trainium2 (Bass/Tile).

8 experts, 8 NeuronCores, one expert per core (SPMD, no collectives).
Per core: out = gelu(x @ w1) @ w2 with
  x [2048, 1024] f32, w1 [1024, 4096] f32, w2 [4096, 1024] f32.

Design (v3 baseline 546.4 us -> ~542 us; the kernel is at the HW roofline):
  - HW ablations showed the stream is purely PE-rate bound. With all 8 cores
    active the PE sustains only ~1.95 GHz (P0 power downclock; the same
    program runs 216 ns/MM = 2.4 GHz on one active core, 265 ns/MM on 8).
    2048 N=512 matmuls x ~265 ns ~= 542 us is the power-limited floor;
    PE busy is ~100% (1-core run: 443 us vs 437 us theoretical peak).
  - All inputs host-prepped into exact SBUF tile layouts: w1/w2 bf16, x
    fp8e3 (e3m4; one-sided quantization of GEMM1's moving operand costs
    ~1.4% rel err vs the 2e-2 gate and trims stream bytes/power). Every
    device DMA is a plain contiguous HWDGE copy; no on-device casts or
    transposes. Output leaves as bf16 outT [1024, 2048]; host finishes.
  - Both GEMMs weight-stationary with the full 2048-token dim moving:
    each stationary tile feeds 4 matmuls (4 token spans of 512 -> 4 PSUM
    banks). A BIR post-pass deletes the redundant Ldweights (walrus emits
    LDW+MM 1:1 with ldweights:false on the MM), so 2048 MMs carry only 512
    weight loads, the structural minimum (contraction_tiles x out_tiles).
  - GEMM1: for j in 32: chain k=0..7 on stationary w1[k,j], moving
    xT[k, tok]; GELU evicts 4 banks -> resident ht[j] (bf16, n-major,
    128 KB/partition). GEMM2: for d in 8: chain j=0..31 on stationary
    w2[j,d], moving ht[j] -> outT staged bf16 + SWDGE store (keeps the
    sync HWDGE ring weights-only so the next rep's w1 is never queued
    behind an out-store).
  - Streams: sync ring w1 (8 chunks) + w2 (8 chunks); scalar ring xt;
    gpsimd out. DMA is fully hidden (noreload ablation == full time).
"""

import os
import sys

import numpy as np

if os.path.isdir("/opt/trn_rl_repo") and "/opt/trn_rl_repo" not in sys.path:
    sys.path.insert(0, "/opt/trn_rl_repo")

NUM_EXPERTS = 8
TOK = 2048
HID = 1024
INT = 4096
OUT = 1024
P = 128

KT = HID // P        # 8 k-tiles (GEMM1 contraction)
JT = INT // P        # 32 j-tiles (intermediate)
DT = OUT // P        # 8 d-tiles (output)
NS = TOK // 512      # 4 moving token spans of 512

W1CJ = 4             # j-tiles per streamed w1 chunk
W1C = JT // W1CJ     # 8 w1 chunks per rep

# GEMM1's moving operand (xT) is fp8e3 (e3m4): one-sided quantization costs
# ~1.4% rel err (vs the 2e-2 gate) and halves the moving-stream bytes /
# reduces PE switching power, which matters because the 8-core kernel is
# P0-power-downclock-bound (~2.0 GHz vs 2.4 single-core).
X_FP8 = True

_PROGRAM_CACHE = {}

# ---------------------------------------------------------------------------
# BIR post-passes, applied via a compile hook:
#  1) dedup_ldweights: walrus emits one Ldweights per Matmult (the Matmult
#     itself has ldweights:false). Consecutive Ldweights with identical
#     weight APs reload the same stationary operand; drop the repeats
#     (convert to NoOp when they carry sync_info).
#  2) split_excess_waits: this compiler build encodes at most 1 sem-wait per
#     instruction; move extras onto NoOps placed before the offender.

LDW_DEDUP = True


def _dedup_ldweights(d) -> bool:
    import orjson

    changed = False
    for fn in d.get("functions", []):
        for blk in fn.get("blocks", []):
            last_key = None
            out = []
            for ins in blk.get("instructions", []):
                if ins.get("engine") != "PE":
                    out.append(ins)
                    continue
                op = ins.get("opcode")
                if op == "Ldweights":
                    key = orjson.dumps(
                        (
                            ins.get("ins"),
                            ins.get("tile_position"),
                            ins.get("tile_size"),
                            ins.get("perf_mode"),
                            ins.get("is_transpose"),
                        )
                    )
                    if key == last_key:
                        changed = True
                        sync = ins.get("sync_info") or {}
                        if sync.get("on_wait") or sync.get("on_update"):
                            ins["opcode"] = "NoOp"
                            ins["ins"] = []
                            ins["outs"] = []
                            out.append(ins)
                        # else: drop entirely
                    else:
                        last_key = key
                        out.append(ins)
                elif op in ("Matmult", "NoOp"):
                    out.append(ins)
                else:
                    last_key = None
                    out.append(ins)
            blk["instructions"] = out
    return changed


def _split_excess_waits(d) -> bool:
    changed = False
    for fn in d.get("functions", []):
        for blk in fn.get("blocks", []):
            out = []
            for ins in blk.get("instructions", []):
                sync = ins.get("sync_info")
                waits = (sync or {}).get("on_wait") or []
                cap = 1
                if len(waits) > cap:
                    changed = True
                    extra, keep = waits[:-cap], waits[-cap:]
                    for i in range(len(extra)):
                        out.append({
                            "name": f"{ins['name']}-wsplit{i}",
                            "opcode": "NoOp",
                            "engine": ins["engine"],
                            "ins": [],
                            "outs": [],
                            "debug": ins.get("debug", 0),
                            "sync_info": {"on_update": [], "on_wait": [extra[i]]},
                        })
                    sync["on_wait"] = keep
                out.append(ins)
            blk["instructions"] = out
    return changed


def _bir_postprocess(bir_json: bytes) -> bytes:
    import orjson

    d = orjson.loads(bir_json)
    changed = False
    if LDW_DEDUP:
        changed |= _dedup_ldweights(d)
    changed |= _split_excess_waits(d)
    return orjson.dumps(d) if changed else bir_json


_hook_installed = False


def _install_wait_split_hook():
    global _hook_installed
    if _hook_installed:
        return
    import concourse.bass2jax as bass2jax
    import concourse.bass_utils as bass_utils

    orig = bass_utils.compile_bir_kernel

    def patched(bir_json, tmpdir, neff_name="file.neff"):
        return orig(_bir_postprocess(bir_json), tmpdir, neff_name)

    bass2jax.compile_bir_kernel = patched
    bass_utils.compile_bir_kernel = patched
    _hook_installed = True


# ---------------------------------------------------------------------------
# Host-side input/output prep


def prep_expert(x, w1, w2):
    """Cast one expert's tensors to bf16 in device tile layout.

    xt  [128, KT*2048]: xt[p, kt, t] = x[t, kt*128 + p]
    w1h [128, JT*KT*128]: w1h[p, jt, kt, c] = w1[kt*128 + p, jt*128 + c]
    w2h [128, DT*JT*128]: w2h[p, dt, jt, c] = w2[jt*128 + p, dt*128 + c]
    """
    import ml_dtypes

    bf = ml_dtypes.bfloat16
    if X_FP8:
        xb = np.clip(x, -15.0, 15.0).astype(ml_dtypes.float8_e3m4)
    else:
        xb = x.astype(bf)
    w1b = w1.astype(bf)
    w2b = w2.astype(bf)
    xt = np.ascontiguousarray(
        xb.T.reshape(KT, P, TOK).transpose(1, 0, 2)
    ).reshape(P, KT * TOK)
    w1h = np.ascontiguousarray(
        w1b.reshape(KT, P, JT, P).transpose(1, 2, 0, 3)
    ).reshape(P, JT * KT * P)
    w2h = np.ascontiguousarray(
        w2b.reshape(JT, P, DT, P).transpose(1, 2, 0, 3)
    ).reshape(P, DT * JT * P)
    return {"xt": xt, "w1": w1h, "w2": w2h}


def make_in_maps(x, w1, w2):
    x = np.ascontiguousarray(x, dtype=np.float32)
    w1 = np.ascontiguousarray(w1, dtype=np.float32)
    w2 = np.ascontiguousarray(w2, dtype=np.float32)
    assert x.shape == (NUM_EXPERTS, TOK, HID)
    assert w1.shape == (NUM_EXPERTS, HID, INT)
    assert w2.shape == (NUM_EXPERTS, INT, OUT)
    return [prep_expert(x[e], w1[e], w2[e]) for e in range(NUM_EXPERTS)]


def finish_output(raw):
    """raw [128, DT*2048] bf16 with raw[p, dt, t] = out[t, dt*128 + p]."""
    arr = np.asarray(raw).reshape(P, DT, TOK).transpose(2, 1, 0)
    return np.ascontiguousarray(arr.reshape(TOK, OUT), dtype=np.float32)


# ---------------------------------------------------------------------------
# Program


def build_program(key=None, repeats=1, ablate=None):
    import concourse.bass as bass
    import concourse.tile as tile
    from concourse import mybir

    if ablate is None:
        ablate = ""
    ablate = set(a for a in ablate.split(",") if a)

    f32 = mybir.dt.float32
    bf16 = mybir.dt.bfloat16
    xdt = mybir.dt.float8e3 if X_FP8 else bf16

    nc = bass.Bass()
    xt_h = nc.declare_dram_parameter("xt", [P, KT * TOK], xdt, isOutput=False)
    w1_h = nc.declare_dram_parameter("w1", [P, JT * KT * P], bf16, isOutput=False)
    w2_h = nc.declare_dram_parameter("w2", [P, DT * JT * P], bf16, isOutput=False)
    out_h = nc.declare_dram_parameter("out", [P, DT * TOK], bf16, isOutput=True)

    xt_r = xt_h[:, :].rearrange("p (kt t) -> p kt t", t=TOK)
    w1_r = w1_h[:, :].rearrange("p (jt kt c) -> p jt kt c", kt=KT, c=P)
    w2_r = w2_h[:, :].rearrange("p (dt jt c) -> p dt jt c", jt=JT, c=P)
    out_r = out_h[:, :].rearrange("p (dt t) -> p dt t", t=TOK)

    gelu = mybir.ActivationFunctionType.Gelu

    with tile.TileContext(nc) as tc:
        with (
            tc.tile_pool(name="xtp", bufs=1) as xt_pool,
            tc.tile_pool(name="htp", bufs=1) as ht_pool,
            tc.tile_pool(name="w1p", bufs=2) as w1_pool,
            tc.tile_pool(name="w2p", bufs=2) as w2_pool,
            tc.tile_pool(name="ostg", bufs=2) as ost_pool,
            tc.tile_pool(name="ps", bufs=2, space="PSUM") as ps_pool,
        ):
            pools = (xt_pool, ht_pool, w1_pool, w2_pool, ost_pool, ps_pool)
            pre = {}
            if "nog1" in ablate:
                ht = ht_pool.tile([P, JT, TOK], bf16, name="ht")
                nc.gpsimd.memset(ht[:], 0.25)
                pre["ht"] = ht
            for _rep in range(repeats):
                _emit_rep(nc, f32, bf16, gelu, xt_r, w1_r, w2_r, out_r,
                          pools, ablate, pre)
    return nc


def _emit_rep(nc, f32, bf16, gelu, xt_r, w1_r, w2_r, out_r, pools,
              ablate=frozenset(), pre=None):
    (xt_pool, ht_pool, w1_pool, w2_pool, ost_pool, ps_pool) = pools
    pre = pre or {}
    skip_g1 = "nog1" in ablate
    skip_g2 = "nog2" in ablate

    # ---- GEMM1: ht[j, tok] = gelu(w1.T @ xT) --------------------------
    g1_passes = 2 if "g1x2" in ablate else 1
    if skip_g1:
        ht = pre["ht"]
    else:
      for _g1pass in range(g1_passes):
        ht = ht_pool.tile([P, JT, TOK], bf16, name="ht")
        xdt = xt_r.dtype
        xt = xt_pool.tile([P, KT, TOK], xdt, name="xt")
        # xt load on the sync HWDGE ring, ahead of the w1/w2 chunks: the
        # next rep's xt then prefetches mid-GEMM2. On the scalar ring it sat
        # behind this rep's last out-copy (which retires only at rep end),
        # serializing the 2MB load into the rep boundary.
        nc.sync.dma_start(out=xt[:, :KT // 2, :], in_=xt_r[:, :KT // 2, :])
        nc.sync.dma_start(out=xt[:, KT // 2:, :], in_=xt_r[:, KT // 2:, :])

        w1c = None
        for j in range(JT):
            if j % W1CJ == 0:
                w1c = w1_pool.tile([P, W1CJ, KT, P], bf16, name="w1c")
                nc.sync.dma_start(
                    out=w1c[:],
                    in_=w1_r[:, j:j + W1CJ, :, :],
                )
            jj = j % W1CJ
            hp = ps_pool.tile([P, NS, 512], f32, name="acc")
            for k in range(KT):
                lhs = w1c[:, jj, k, :]
                for s in range(NS):
                    nc.tensor.matmul(
                        hp[:, s, :],
                        lhs,
                        xt[:, k, s * 512:(s + 1) * 512],
                        start=(k == 0),
                        stop=(k == KT - 1),
                    )
            nc.scalar.activation(ht[:, j, :], hp[:, :, :], gelu)

    # ---- GEMM2: outT[d, tok] = ht.T-contract @ w2 ---------------------
    if skip_g2:
        # still evict something so the rep has an output dependency
        ot = ost_pool.tile([P, TOK], bf16, name="ot")
        nc.scalar.copy(ot[:], ht[:, 0, :])
        nc.sync.dma_start(out=out_r[:, 0, :], in_=ot[:])
        return

    for d in range(DT):
        w2c = w2_pool.tile([P, 1, JT, P], bf16, name="w2c")
        nc.sync.dma_start(out=w2c[:], in_=w2_r[:, d:d + 1, :, :])
        op = ps_pool.tile([P, NS, 512], f32, name="acc")
        for j in range(JT):
            lhs = w2c[:, 0, j, :]
            for s in range(NS):
                nc.tensor.matmul(
                    op[:, s, :],
                    lhs,
                    ht[:, j, s * 512:(s + 1) * 512],
                    start=(j == 0),
                    stop=(j == JT - 1),
                )
        if "noout" in ablate and d != DT - 1:
            continue
        ot = ost_pool.tile([P, NS, 512], bf16, name="ot")
        nc.scalar.copy(ot[:], op[:])
        # out-store on the (otherwise idle) SWDGE path: keeps the sync HWDGE
        # ring weights-only, so the next rep's first w1 chunk is not queued
        # behind this rep's last out-store (FIFO per ring).
        nc.gpsimd.dma_start(
            out=out_r[:, d, :],
            in_=ot[:].rearrange("p a b -> p (a b)"),
        )


def _get_program():
    if "v4" not in _PROGRAM_CACHE:
        _PROGRAM_CACHE["v4"] = build_program()
    return _PROGRAM_CACHE["v4"]


def kernel(x, w1, w2, _trace=False, _trace_kwargs=None):
    """Full-tensor entry point: shards experts across 8 cores, returns full out."""
    from concourse.bass_utils import run_bass_kernel_spmd

    _install_wait_split_hook()
    in_maps = make_in_maps(x, w1, w2)
    nc = _get_program()
    core_ids = list(range(NUM_EXPERTS))
    kw = {}
    if _trace:
        kw["trace"] = True
        kw["trace_kwargs"] = _trace_kwargs or {}
    res = run_bass_kernel_spmd(nc, in_maps, core_ids, **kw)
    out = np.stack(
        [finish_output(res.results[e]["out"]) for e in range(NUM_EXPERTS)], axis=0
    )
    if _trace:
        return out, res
    return out


if __name__ == "__main__":
    rng = np.random.default_rng(0)
    x = rng.standard_normal((NUM_EXPERTS, TOK, HID), dtype=np.float32)
    w1 = rng.standard_normal((NUM_EXPERTS, HID, INT), dtype=np.float32) * 0.03
    w2 = rng.standard_normal((NUM_EXPERTS, INT, OUT), dtype=np.float32) * 0.015
    out = kernel(x, w1, w2)
    print("out", out.shape, out.dtype, float(np.abs(out).mean()))


# revision 10
# speedup vs baseline: 1.0091x; 1.0017x over previous
"""MoE expert-parallel kernel v5 for Trainium2 (Bass/Tile).

8 experts, 8 NeuronCores, one expert per core (SPMD, no collectives).
Per core: out = gelu(x @ w1) @ w2 with
  x [2048, 1024] f32, w1 [1024, 4096] f32, w2 [4096, 1024] f32.

Design (v3 baseline 546.4 us -> ~542 us; the kernel is at the HW roofline):
  - HW ablations showed the stream is purely PE-rate bound. With all 8 cores
    active the PE sustains only ~1.95 GHz (P0 power downclock; the same
    program runs 216 ns/MM = 2.4 GHz on one active core, 265 ns/MM on 8).
    2048 N=512 matmuls x ~265 ns ~= 542 us is the power-limited floor;
    PE busy is ~100% (1-core run: 443 us vs 437 us theoretical peak).
  - All inputs host-prepped into exact SBUF tile layouts: w1/w2 bf16, x
    fp8e3 (e3m4; one-sided quantization of GEMM1's moving operand costs
    ~1.4% rel err vs the 2e-2 gate and trims stream bytes/power). Every
    device DMA is a plain contiguous HWDGE copy; no on-device casts or
    transposes. Output leaves as bf16 outT [1024, 2048]; host finishes.
  - Both GEMMs weight-stationary with the full 2048-token dim moving:
    each stationary tile feeds 4 matmuls (4 token spans of 512 -> 4 PSUM
    banks). A BIR post-pass deletes the redundant Ldweights (walrus emits
    LDW+MM 1:1 with ldweights:false on the MM), so 2048 MMs carry only 512
    weight loads, the structural minimum (contraction_tiles x out_tiles).
  - GEMM1: for j in 32: chain k=0..7 on stationary w1[k,j], moving
    xT[k, tok]; GELU evicts 4 banks -> resident ht[j] (bf16, n-major,
    128 KB/partition). GEMM2: for d in 8: chain j=0..31 on stationary
    w2[j,d], moving ht[j] -> outT staged bf16 + SWDGE store (keeps the
    sync HWDGE ring weights-only so the next rep's w1 is never queued
    behind an out-store).
  - Streams: sync ring w1 (8 chunks) + w2 (8 chunks); scalar ring xt;
    gpsimd out. DMA is fully hidden (noreload ablation == full time).
"""

import os
import sys

import numpy as np

if os.path.isdir("/opt/trn_rl_repo") and "/opt/trn_rl_repo" not in sys.path:
    sys.path.insert(0, "/opt/trn_rl_repo")

NUM_EXPERTS = 8
TOK = 2048
HID = 1024
INT = 4096
OUT = 1024
P = 128

KT = HID // P        # 8 k-tiles (GEMM1 contraction)
JT = INT // P        # 32 j-tiles (intermediate)
DT = OUT // P        # 8 d-tiles (output)
SW = 512             # moving-span width; one MM output = one PSUM bank
                     # (512 f32) is a hard ISA limit (s3d3_mm_num_elements:
                     # N=1024 fails walrus codegen; the "128x1024 bf16/fp8
                     # moving max" applies to DoubleRow's paired input only)
NS = TOK // SW       # 4 moving token spans per stationary tile

W1CJ = 4             # j-tiles per streamed w1 chunk
W1C = JT // W1CJ     # 8 w1 chunks per rep

# GEMM1's moving operand (xT) is fp8e3 (e3m4): one-sided quantization costs
# ~1.4% rel err (vs the 2e-2 gate) and halves the moving-stream bytes /
# reduces PE switching power, which matters because the 8-core kernel is
# P0-power-downclock-bound (~2.0 GHz vs 2.4 single-core).
X_FP8 = True

_PROGRAM_CACHE = {}

# ---------------------------------------------------------------------------
# BIR post-passes, applied via a compile hook:
#  1) dedup_ldweights: walrus emits one Ldweights per Matmult (the Matmult
#     itself has ldweights:false). Consecutive Ldweights with identical
#     weight APs reload the same stationary operand; drop the repeats
#     (convert to NoOp when they carry sync_info).
#  2) split_excess_waits: this compiler build encodes at most 1 sem-wait per
#     instruction; move extras onto NoOps placed before the offender.

LDW_DEDUP = True


def _dedup_ldweights(d) -> bool:
    import orjson

    changed = False
    for fn in d.get("functions", []):
        for blk in fn.get("blocks", []):
            last_key = None
            out = []
            for ins in blk.get("instructions", []):
                if ins.get("engine") != "PE":
                    out.append(ins)
                    continue
                op = ins.get("opcode")
                if op == "Ldweights":
                    key = orjson.dumps(
                        (
                            ins.get("ins"),
                            ins.get("tile_position"),
                            ins.get("tile_size"),
                            ins.get("perf_mode"),
                            ins.get("is_transpose"),
                        )
                    )
                    if key == last_key:
                        changed = True
                        sync = ins.get("sync_info") or {}
                        if sync.get("on_wait") or sync.get("on_update"):
                            ins["opcode"] = "NoOp"
                            ins["ins"] = []
                            ins["outs"] = []
                            out.append(ins)
                        # else: drop entirely
                    else:
                        last_key = key
                        out.append(ins)
                elif op in ("Matmult", "NoOp"):
                    out.append(ins)
                else:
                    last_key = None
                    out.append(ins)
            blk["instructions"] = out
    return changed


def _split_excess_waits(d) -> bool:
    changed = False
    for fn in d.get("functions", []):
        for blk in fn.get("blocks", []):
            out = []
            for ins in blk.get("instructions", []):
                sync = ins.get("sync_info")
                waits = (sync or {}).get("on_wait") or []
                cap = 1
                if len(waits) > cap:
                    changed = True
                    extra, keep = waits[:-cap], waits[-cap:]
                    for i in range(len(extra)):
                        out.append({
                            "name": f"{ins['name']}-wsplit{i}",
                            "opcode": "NoOp",
                            "engine": ins["engine"],
                            "ins": [],
                            "outs": [],
                            "debug": ins.get("debug", 0),
                            "sync_info": {"on_update": [], "on_wait": [extra[i]]},
                        })
                    sync["on_wait"] = keep
                out.append(ins)
            blk["instructions"] = out
    return changed


def _bir_postprocess(bir_json: bytes) -> bytes:
    import orjson

    d = orjson.loads(bir_json)
    changed = False
    if LDW_DEDUP:
        changed |= _dedup_ldweights(d)
    changed |= _split_excess_waits(d)
    return orjson.dumps(d) if changed else bir_json


_hook_installed = False


def _install_wait_split_hook():
    global _hook_installed
    if _hook_installed:
        return
    import concourse.bass2jax as bass2jax
    import concourse.bass_utils as bass_utils

    orig = bass_utils.compile_bir_kernel

    def patched(bir_json, tmpdir, neff_name="file.neff"):
        return orig(_bir_postprocess(bir_json), tmpdir, neff_name)

    bass2jax.compile_bir_kernel = patched
    bass_utils.compile_bir_kernel = patched
    _hook_installed = True


# ---------------------------------------------------------------------------
# Host-side input/output prep


def prep_expert(x, w1, w2):
    """Cast one expert's tensors to bf16 in device tile layout.

    xt  [128, KT*2048]: xt[p, kt, t] = x[t, kt*128 + p]
    w1h [128, JT*KT*128]: w1h[p, jt, kt, c] = w1[kt*128 + p, jt*128 + c]
    w2h [128, DT*JT*128]: w2h[p, dt, jt, c] = w2[jt*128 + p, dt*128 + c]
    """
    import ml_dtypes

    bf = ml_dtypes.bfloat16
    if X_FP8:
        xb = np.clip(x, -15.0, 15.0).astype(ml_dtypes.float8_e3m4)
    else:
        xb = x.astype(bf)
    w1b = w1.astype(bf)
    w2b = w2.astype(bf)
    xt = np.ascontiguousarray(
        xb.T.reshape(KT, P, TOK).transpose(1, 0, 2)
    ).reshape(P, KT * TOK)
    w1h = np.ascontiguousarray(
        w1b.reshape(KT, P, JT, P).transpose(1, 2, 0, 3)
    ).reshape(P, JT * KT * P)
    w2h = np.ascontiguousarray(
        w2b.reshape(JT, P, DT, P).transpose(1, 2, 0, 3)
    ).reshape(P, DT * JT * P)
    return {"xt": xt, "w1": w1h, "w2": w2h}


def make_in_maps(x, w1, w2):
    x = np.ascontiguousarray(x, dtype=np.float32)
    w1 = np.ascontiguousarray(w1, dtype=np.float32)
    w2 = np.ascontiguousarray(w2, dtype=np.float32)
    assert x.shape == (NUM_EXPERTS, TOK, HID)
    assert w1.shape == (NUM_EXPERTS, HID, INT)
    assert w2.shape == (NUM_EXPERTS, INT, OUT)
    return [prep_expert(x[e], w1[e], w2[e]) for e in range(NUM_EXPERTS)]


def finish_output(raw):
    """raw [128, DT*2048] bf16 with raw[p, dt, t] = out[t, dt*128 + p]."""
    arr = np.asarray(raw).reshape(P, DT, TOK).transpose(2, 1, 0)
    return np.ascontiguousarray(arr.reshape(TOK, OUT), dtype=np.float32)


# ---------------------------------------------------------------------------
# Program


def build_program(key=None, repeats=1, ablate=None):
    import concourse.bass as bass
    import concourse.tile as tile
    from concourse import mybir

    if ablate is None:
        ablate = ""
    ablate = set(a for a in ablate.split(",") if a)

    f32 = mybir.dt.float32
    bf16 = mybir.dt.bfloat16
    xdt = mybir.dt.float8e3 if X_FP8 else bf16

    nc = bass.Bass()
    xt_h = nc.declare_dram_parameter("xt", [P, KT * TOK], xdt, isOutput=False)
    w1_h = nc.declare_dram_parameter("w1", [P, JT * KT * P], bf16, isOutput=False)
    w2_h = nc.declare_dram_parameter("w2", [P, DT * JT * P], bf16, isOutput=False)
    out_h = nc.declare_dram_parameter("out", [P, DT * TOK], bf16, isOutput=True)

    xt_r = xt_h[:, :].rearrange("p (kt t) -> p kt t", t=TOK)
    w1_r = w1_h[:, :].rearrange("p (jt kt c) -> p jt kt c", kt=KT, c=P)
    w2_r = w2_h[:, :].rearrange("p (dt jt c) -> p dt jt c", jt=JT, c=P)
    out_r = out_h[:, :].rearrange("p (dt t) -> p dt t", t=TOK)

    gelu = mybir.ActivationFunctionType.Gelu

    with tile.TileContext(nc) as tc:
        with (
            tc.tile_pool(name="xtp", bufs=1) as xt_pool,
            tc.tile_pool(name="htp", bufs=1) as ht_pool,
            tc.tile_pool(name="w1p", bufs=2) as w1_pool,
            tc.tile_pool(name="w2p", bufs=2) as w2_pool,
            tc.tile_pool(name="ostg", bufs=2) as ost_pool,
            tc.tile_pool(name="ps", bufs=2, space="PSUM") as ps_pool,
        ):
            pools = (xt_pool, ht_pool, w1_pool, w2_pool, ost_pool, ps_pool)
            pre = {}
            if "nog1" in ablate:
                ht = ht_pool.tile([P, JT, TOK], bf16, name="ht")
                nc.gpsimd.memset(ht[:], 0.25)
                pre["ht"] = ht
            for _rep in range(repeats):
                _emit_rep(nc, f32, bf16, gelu, xt_r, w1_r, w2_r, out_r,
                          pools, ablate, pre)
    return nc


def _emit_rep(nc, f32, bf16, gelu, xt_r, w1_r, w2_r, out_r, pools,
              ablate=frozenset(), pre=None):
    (xt_pool, ht_pool, w1_pool, w2_pool, ost_pool, ps_pool) = pools
    pre = pre or {}
    skip_g1 = "nog1" in ablate
    skip_g2 = "nog2" in ablate

    # ---- GEMM1: ht[j, tok] = gelu(w1.T @ xT) --------------------------
    g1_passes = 2 if "g1x2" in ablate else 1
    if skip_g1:
        ht = pre["ht"]
    else:
      for _g1pass in range(g1_passes):
        ht = ht_pool.tile([P, JT, TOK], bf16, name="ht")
        xdt = xt_r.dtype
        xt = xt_pool.tile([P, KT, TOK], xdt, name="xt")
        # xt load on the sync HWDGE ring, ahead of the w1/w2 chunks: the
        # next rep's xt then prefetches mid-GEMM2. On the scalar ring it sat
        # behind this rep's last out-copy (which retires only at rep end),
        # serializing the 2MB load into the rep boundary.
        nc.sync.dma_start(out=xt[:, :KT // 2, :], in_=xt_r[:, :KT // 2, :])
        nc.sync.dma_start(out=xt[:, KT // 2:, :], in_=xt_r[:, KT // 2:, :])

        w1c = None
        for j in range(JT):
            if j % W1CJ == 0:
                w1c = w1_pool.tile([P, W1CJ, KT, P], bf16, name="w1c")
                nc.sync.dma_start(
                    out=w1c[:],
                    in_=w1_r[:, j:j + W1CJ, :, :],
                )
            jj = j % W1CJ
            hp = ps_pool.tile([P, NS, SW], f32, name="acc")
            for k in range(KT):
                lhs = w1c[:, jj, k, :]
                for s in range(NS):
                    nc.tensor.matmul(
                        hp[:, s, :],
                        lhs,
                        xt[:, k, s * SW:(s + 1) * SW],
                        start=(k == 0),
                        stop=(k == KT - 1),
                    )
            nc.scalar.activation(ht[:, j, :], hp[:, :, :], gelu)

    # ---- GEMM2: outT[d, tok] = ht.T-contract @ w2 ---------------------
    if skip_g2:
        # still evict something so the rep has an output dependency
        ot = ost_pool.tile([P, TOK], bf16, name="ot")
        nc.scalar.copy(ot[:], ht[:, 0, :])
        nc.sync.dma_start(out=out_r[:, 0, :], in_=ot[:])
        return

    for d in range(DT):
        w2c = w2_pool.tile([P, 1, JT, P], bf16, name="w2c")
        nc.sync.dma_start(out=w2c[:], in_=w2_r[:, d:d + 1, :, :])
        op = ps_pool.tile([P, NS, SW], f32, name="acc")
        for j in range(JT):
            lhs = w2c[:, 0, j, :]
            for s in range(NS):
                nc.tensor.matmul(
                    op[:, s, :],
                    lhs,
                    ht[:, j, s * SW:(s + 1) * SW],
                    start=(j == 0),
                    stop=(j == JT - 1),
                )
        if "noout" in ablate and d != DT - 1:
            continue
        ot = ost_pool.tile([P, NS, SW], bf16, name="ot")
        nc.scalar.copy(ot[:], op[:])
        # out-store on the (otherwise idle) SWDGE path: keeps the sync HWDGE
        # ring weights-only, so the next rep's first w1 chunk is not queued
        # behind this rep's last out-store (FIFO per ring).
        nc.gpsimd.dma_start(
            out=out_r[:, d, :],
            in_=ot[:].rearrange("p a b -> p (a b)"),
        )


def _get_program():
    if "v4" not in _PROGRAM_CACHE:
        _PROGRAM_CACHE["v4"] = build_program()
    return _PROGRAM_CACHE["v4"]


def kernel(x, w1, w2, _trace=False, _trace_kwargs=None):
    """Full-tensor entry point: shards experts across 8 cores, returns full out."""
    from concourse.bass_utils import run_bass_kernel_spmd

    _install_wait_split_hook()
    in_maps = make_in_maps(x, w1, w2)
    nc = _get_program()
    core_ids = list(range(NUM_EXPERTS))
    kw = {}
    if _trace:
        kw["trace"] = True
        kw["trace_kwargs"] = _trace_kwargs or {}
    res = run_bass_kernel_spmd(nc, in_maps, core_ids, **kw)
    out = np.stack(
        [finish_output(res.results[e]["out"]) for e in range(NUM_EXPERTS)], axis=0
    )
    if _trace:
        return out, res
    return out


if __name__ == "__main__":
    rng = np.random.default_rng(0)
    x = rng.standard_normal((NUM_EXPERTS, TOK, HID), dtype=np.float32)
    w1 = rng.standard_normal((NUM_EXPERTS, HID, INT), dtype=np.float32) * 0.03
    w2 = rng.standard_normal((NUM_EXPERTS, INT, OUT), dtype=np.float32) * 0.015
    out = kernel(x, w1, w2)
    print("out", out.shape, out.dtype, float(np.abs(out).mean()))
